# revision 19
# baseline (speedup 1.0000x reference)
"""Masked multi-head attention on 8 Trainium2 NeuronCores.

Problem (hardcoded): x[4,2048,512] f32, mask[1,4,2048,2048] bool,
Wq/Wk/Wv[512,512] f32.  out = softmax(mask? -inf : (xWq.T)(xWk.T).T/sqrt(128)) @ (xWv.T)
per head (8 heads of dim 64), merged back to [4,2048,512] f32.

Sharding: core c handles batch b=c//2 and head-quad hg=c%2 (heads hg*4..hg*4+3).
Scores are built in "ST" layout [k_partitions, q_free] so the PV matmul needs
no transposes, softmax sums ride free as a ones-column appended to V, and the
final [q, d] layout is produced with PE transposes before normalization.
The boolean mask is applied multiplicatively after exp (notmask DMA'd as bf16
0/1, one DVE tensor_mul per head tile); optionally a fraction of kt tiles
("injf<N>" ablation) instead add MASKBIG*nm to scores via a PE identity-matmul
and fold -MASKBIG/TEMP into the exp bias (numerically exact, moves mask work
DVE->PE). Projections are emitted as fine-grained tasks interleaved into the
attention slots so ACT/DVE start early.

Epilogue (v2): per (qb, m) the [65,1024] PV accumulator is copied once to
SBUF, transposed with 8 PE transposes into ONE 2-bank PSUM tile (4 slices per
bank — a slice must never straddle a 512-f32 bank line), then one batched
reciprocal + one broadcast tensor_mul + one DMA. Q/K/V pools double-buffered
(qkvp bufs=2) so consecutive bodies overlap and the ACT exp stream (the ~109us
roofline: 128 exp tiles x ~855ns, 1 elem/lane/cycle @1.2GHz) stays dense.

PSUM (v3): projections get their OWN 2-slot 1-bank pool (projp) instead of
sharing the score-st slots; banks = st 2x2 + projp 2x1 + ot 1x2 = 8. Measured
med -18us vs the shared 3-slot layout (proj groups were starving the
score->exp pipeline). "pshare" ablation reverts.

All matmul operands are bf16 with f32 PSUM accumulation; measured end-to-end
rel-err vs the f32 reference is ~4e-3.
"""

from collections import deque

import numpy as np
import ml_dtypes

import concourse.bass as bass
import concourse.mybir as mybir
import concourse.tile as tile
from concourse import bacc
from concourse.bass_utils import run_bass_kernel_spmd
from concourse.masks import make_identity

BF16 = mybir.dt.bfloat16
F32 = mybir.dt.float32
NPBF16 = ml_dtypes.bfloat16

B, N, C = 4, 2048, 512
H, D = 8, 64
TEMP = float((2.0 * D) ** 0.5)  # sqrt(128)
P = 128
NCORES = 8
HPC = H // 2          # 4 heads per core
DQ = HPC * D          # 256 projection cols per core
KT = N // P           # 16 k tiles
QB = N // 512         # 4 q blocks
VW = D + 1            # V width incl. ones column
AV_DEPTH = 2          # software-pipeline depth for PV matmul emission
MASKBIG = 352.0       # PE-injected mask amplitude; exp(-MASKBIG/TEMP) ~ 3e-14


def _build_program(repeat=1, ablate=()):
    """ablate: timing-only ablations ('nomask','halfexp','noav','unpackst')."""
    nc = bacc.Bacc(
        "TRN2",
        target_bir_lowering=False,
        debug=False,
        enable_asserts=False,
        num_devices=NCORES,
    )

    xT = nc.dram_tensor("xT", [C, N], BF16, kind="ExternalInput").ap()
    wqT = nc.dram_tensor("wqT", [C, DQ], BF16, kind="ExternalInput").ap()
    wkT = nc.dram_tensor("wkT", [C, DQ], BF16, kind="ExternalInput").ap()
    wvT = nc.dram_tensor("wvT", [C, DQ], BF16, kind="ExternalInput").ap()
    nmT = nc.dram_tensor("nmT", [N, N], BF16, kind="ExternalInput").ap()
    o = nc.dram_tensor("o", [N, DQ], F32, kind="ExternalOutput").ap()

    nm_view = nmT.rearrange("(t p) q -> p t q", p=P)  # [128, 16, 2048]

    with tile.TileContext(nc) as tc:
        with (
            tc.tile_pool(name="constp", bufs=1) as constp,
            tc.tile_pool(name="xp", bufs=1 if "xp1" in ablate else 2) as xp,
            tc.tile_pool(name="wp", bufs=1) as wp,
            tc.tile_pool(name="qkvp", bufs=1 if "qkv1" in ablate else 2) as qkvp,
            tc.tile_pool(name="maskp", bufs=10 if "mchunk" in ablate else (2 if "tight" in ablate else (4 if "slack2" in ablate else 3))) as maskp,
            tc.tile_pool(name="workp", bufs=6 if "tight" in ablate else (10 if "slack2" in ablate else 8)) as workp,
            tc.tile_pool(name="outp", bufs=3 if "outp3" in ablate else 4) as outp,
            tc.tile_pool(name="psp",
                         bufs=3 if "pshare" in ablate else 2,
                         space="PSUM") as psp,
            tc.tile_pool(name="projp", bufs=1 if "pshare" in ablate else 2,
                         space="PSUM") as projp,
            tc.tile_pool(name="psot", bufs=2 if "bufs2" in ablate else 1,
                         space="PSUM") as psot,
        ):
            ident = constp.tile([P, P], F32)
            make_identity(nc, ident)
            ident_bf = constp.tile([P, P], BF16)
            make_identity(nc, ident_bf)
            ident_big = constp.tile([P, P], BF16)
            make_identity(nc, ident_big)
            nc.vector.tensor_scalar_mul(ident_big, ident_big, MASKBIG)
            ebias_t = constp.tile([P, 1], F32)
            nc.vector.memset(ebias_t, -MASKBIG / TEMP)
            for _ in range(repeat):
                _emit_body(nc, tc, xT, wqT, wkT, wvT, nm_view, o,
                           xp, wp, qkvp, maskp, workp, outp, psp, projp,
                           psot, ident, ident_bf, ident_big, ebias_t,
                           ablate=ablate)

    nc.compile()
    return nc


def _emit_body(nc, tc, xT, wqT, wkT, wvT, nm_view, o,
               xp, wp, qkvp, maskp, workp, outp, psp, projp, psot, ident,
               ident_bf, ident_big, ebias_t, ablate=()):
    # ---- load inputs ----
    xt = []
    for c in range(4):
        t = xp.tile([P, N], BF16, name=f"xt{c}", tag=f"xt{c}")
        eng = nc.gpsimd if ("dmaq" in ablate and c % 2) else nc.sync
        eng.dma_start(out=t, in_=xT[c * P:(c + 1) * P, :])
        xt.append(t)
    ws = {}
    for wname, wdram in (("q", wqT), ("k", wkT), ("v", wvT)):
        chunks = []
        for c in range(4):
            t = wp.tile([P, DQ], BF16, name=f"w{wname}{c}", tag=f"w{wname}{c}")
            nc.sync.dma_start(out=t, in_=wdram[c * P:(c + 1) * P, :])
            chunks.append(t)
        ws[wname] = chunks

    # ---- projections ----
    # QT/KT in [d', n] layout: partition tile m holds heads (2m, 2m+1).
    qt_sb = [qkvp.tile([P, N], BF16, name=f"qt_sb{m}", tag=f"qt{m}") for m in range(2)]
    kt_sb = [qkvp.tile([P, N], BF16, name=f"kt_sb{m}", tag=f"kt{m}") for m in range(2)]

    def qk_group(wname, m, nb):
        t = (qt_sb if wname == "q" else kt_sb)[m]
        if "pshare" in ablate:
            ps = psp.tile([P, 512], F32, name="proj_ps", tag="st")
        else:
            ps = projp.tile([P, 512], F32, name="proj_ps", tag="pj")
        for c in range(4):
            nc.tensor.matmul(
                ps,
                lhsT=ws[wname][c][:, m * P:(m + 1) * P],
                rhs=xt[c][:, nb * 512:(nb + 1) * 512],
                start=(c == 0),
                stop=(c == 3),
            )
        if "projact" in ablate:
            nc.scalar.copy(t[:, nb * 512:(nb + 1) * 512], ps)
        else:
            nc.vector.tensor_copy(t[:, nb * 512:(nb + 1) * 512], ps)

    # V in [k, d'] layout with a ones column per head: [128, kt*(4*65)]
    vext = qkvp.tile([P, KT * HPC * VW], BF16)
    nc.gpsimd.memset(vext, 1.0)

    def v_group(kti):
        if "pshare" in ablate:
            ps = psp.tile([P, DQ], F32, name="v_ps", tag="st")
        else:
            ps = projp.tile([P, DQ], F32, name="v_ps", tag="pj")
        for c in range(4):
            nc.tensor.matmul(
                ps,
                lhsT=xt[c][:, kti * P:(kti + 1) * P],
                rhs=ws["v"][c],
                start=(c == 0),
                stop=(c == 3),
            )
        dst_view = vext[:, kti * HPC * VW:(kti + 1) * HPC * VW].rearrange(
            "p (h e) -> p h e", h=HPC
        )[:, :, 0:D]
        src_view = ps.rearrange("p (h e) -> p h e", h=HPC)
        nc.vector.tensor_copy(dst_view, src_view)

    # minimal prelude: only what the first (qb0, m0) scores need right away
    qk_group("q", 0, 0)
    qk_group("k", 0, 0)
    # remaining projection work, drained one task per kt slot (deadline-safe)
    prelude = deque()
    for spec in [("k", 0, 1), ("k", 0, 2), ("k", 0, 3),
                 ("q", 1, 0), ("k", 1, 0), ("k", 1, 1), ("k", 1, 2), ("k", 1, 3),
                 ("q", 1, 1), ("q", 1, 2), ("q", 1, 3),
                 ("q", 0, 1), ("q", 0, 2), ("q", 0, 3)]:
        prelude.append(lambda spec=spec: qk_group(*spec))
    vqueue = deque(lambda kti=kti: v_group(kti) for kti in range(KT))

    # ---- attention (software-pipelined emission) ----
    av_queue = deque()   # deferred PV-matmul emissions
    epi_stages = deque() # deferred epilogue stages of the previous (m, qb)

    def emit_slot():
        """Emit one deferred AV (if the pipeline is full) and one epilogue stage."""
        if vqueue:
            vqueue.popleft()()
        if prelude:
            prelude.popleft()()
        if len(av_queue) > (AV_DEPTH if "tight" in ablate else (4 if "slack2" in ablate else 3)):
            av_queue.popleft()()
        if epi_stages:
            epi_stages.popleft()()

    def make_epilogue(ot, m, qb):
        # v2: numerators+sums copied once to SBUF (bf16), transposed with 8
        # cheap FWL bf16 matmuls into ONE [128, 8*65] PSUM tile, then a single
        # batched reciprocal + one broadcast tensor_mul + one DMA per (qb, m).
        stages = []

        def copy_stage():
            ots = outp.tile([VW, 1024], F32, name="ots", tag="ots")
            if "otact" in ablate:
                nc.scalar.copy(ots, ot)
            else:
                nc.vector.tensor_copy(ots, ot)
            stages.append(ots)  # [0]
        yield copy_stage

        def tr_stage(half):
            def f():
                ots = stages[0]
                if "trp" in ablate:
                    # 1-bank half in the (mostly idle) projection pool: keeps
                    # BOTH score-st slots free during epilogues
                    tr = projp.tile([P, 4 * VW], F32, name="trh", tag="pj")
                    stages.append(tr)  # [1], [2]
                else:
                    if half == 0:
                        tr = psp.tile([P, 1024], F32, name="tr", tag="st")
                        stages.append(tr)  # [1]
                    tr = stages[1]
                for j in range(half * 4, half * 4 + 4):
                    sl, hl = j // 2, j % 2
                    # 4 slices per PSUM bank so no slice straddles a bank line
                    if "trp" in ablate:
                        col = (j % 4) * VW
                    else:
                        col = (j // 4) * 512 + (j % 4) * VW
                    nc.tensor.transpose(
                        tr[:, col:col + VW],
                        ots[:, hl * 512 + sl * P: hl * 512 + (sl + 1) * P],
                        ident[0:VW, 0:VW],
                    )
            return f
        yield tr_stage(0)
        yield tr_stage(1)

        def norm_stage():
            ob = outp.tile([P, 8 * D], F32, name="ob", tag="ob")
            rec = outp.tile([P, 8], F32, name="rec", tag="rec")
            if "trp" in ablate:
                for g in range(2):
                    trv = stages[1 + g].rearrange("p (j e) -> p j e", j=4)
                    recv = rec[:, g * 4:(g + 1) * 4]
                    nc.vector.reciprocal(recv[:, :, None], trv[:, :, D:D + 1])
                    obv = ob[:, g * 4 * D:(g + 1) * 4 * D] \
                        .rearrange("p (j e) -> p j e", j=4)
                    recb = recv[:, :, None].broadcast_to([P, 4, D])
                    nc.vector.tensor_mul(obv, trv[:, :, 0:D], recb)
            else:
                tr = stages[1]
                trv = tr.rearrange("p (g je) -> p g je", g=2)[:, :, 0:4 * VW] \
                        .rearrange("p g (j e) -> p g j e", j=4)
                recv = rec.rearrange("p (g j) -> p g j", g=2)
                nc.vector.reciprocal(recv[:, :, :, None], trv[:, :, :, D:D + 1])
                obv = ob.rearrange("p (g j e) -> p g j e", g=2, e=D)
                recb = recv[:, :, :, None].broadcast_to([P, 2, 4, D])
                nc.vector.tensor_mul(obv, trv[:, :, :, 0:D], recb)
            stages.append(ob)
        yield norm_stage

        def dma_stage():
            ob = stages[-1]
            nc.sync.dma_start(
                out=o[qb * 512:(qb + 1) * 512, 2 * m * D:(2 * m + 2) * D]
                    .rearrange("(sl p) (hl d) -> p sl hl d", p=P, hl=2),
                in_=ob.rearrange("p (sl hl d) -> p sl hl d", sl=4, hl=2),
            )
        yield dma_stage

    for qb in range(QB):
        nm_eng = nc.gpsimd if "dmaq" in ablate else nc.sync
        if "mchunk" in ablate:
            nms = []
            for ch in range(4):
                t = maskp.tile([P, 4, 512], BF16, name="nm", tag="nm")
                nm_eng.dma_start(
                    out=t,
                    in_=nm_view[:, ch * 4:(ch + 1) * 4,
                                qb * 512:(qb + 1) * 512])
                nms.append(t)

            def nm_at(kti):
                return nms[kti // 4][:, kti % 4, :]
        else:
            nm = maskp.tile([P, KT, 512], BF16, name="nm", tag="nm")
            nm_eng.dma_start(out=nm, in_=nm_view[:, :, qb * 512:(qb + 1) * 512])

            def nm_at(kti):
                return nm[:, kti, :]
        for m in range(2):
            ot = psot.tile([VW, 1024], F32, name="ot", tag="ot")
            for kti in range(KT):
                st = psp.tile([P, 1024], F32, name="st", tag="st")
                inject = "inject" in ablate
                injn = next((int(a[4:]) for a in ablate
                             if a.startswith("injf")), 0)
                injected = injn and (kti % injn == injn - 1)
                for hl in range(2):
                    nc.tensor.matmul(
                        st[:, hl * 512:(hl + 1) * 512],
                        lhsT=kt_sb[m][hl * D:(hl + 1) * D, kti * P:(kti + 1) * P],
                        rhs=qt_sb[m][hl * D:(hl + 1) * D, qb * 512:(qb + 1) * 512],
                        start=True,
                        stop=not (inject or injected),
                    )
                if inject or injected:
                    big = ident_bf if inject else ident_big
                    for hl in range(2):
                        nc.tensor.matmul(
                            st[:, hl * 512:(hl + 1) * 512],
                            lhsT=big[:, :],
                            rhs=nm_at(kti),
                            start=False,
                            stop=True,
                        )
                ex = workp.tile([P, 1024], BF16, name="ex", tag="ex")
                ebias = ebias_t if injected else 0.0
                if "halfexp" in ablate:
                    nc.scalar.activation(
                        ex[:, 0:512], st[:, 0:512],
                        mybir.ActivationFunctionType.Exp, scale=1.0 / TEMP
                    )
                else:
                    nc.scalar.activation(
                        ex, st, mybir.ActivationFunctionType.Exp,
                        scale=1.0 / TEMP, bias=ebias,
                    )
                if "nomask" not in ablate and "inject" not in ablate \
                        and not injected:
                    if "bmask" in ablate:
                        exv = ex.rearrange("p (t q) -> p t q", t=2)
                        nmb = nm_at(kti)[:, None, :].broadcast_to([P, 2, 512])
                        nc.vector.tensor_mul(exv, exv, nmb)
                    else:
                        gpn = next((int(a[2:]) for a in ablate
                                    if a.startswith("gp") and a[2:].isdigit()),
                                   0)
                        eng = (nc.gpsimd if gpn and kti % gpn == gpn - 1
                               else nc.vector)
                        for hl in range(2):
                            eng.tensor_mul(
                                ex[:, hl * 512:(hl + 1) * 512],
                                ex[:, hl * 512:(hl + 1) * 512],
                                nm_at(kti),
                            )

                def av_stage(ot=ot, ex=ex, kti=kti, m=m, qb=qb):
                    if "noav" in ablate:
                        if kti == 0:
                            nc.vector.memset(ot, 1.0)
                        if kti == KT - 1:
                            epi_stages.extend(make_epilogue(ot, m, qb))
                        return
                    for hl in range(2):
                        h = 2 * m + hl
                        nc.tensor.matmul(
                            ot[:, hl * 512:(hl + 1) * 512],
                            lhsT=vext[:, (kti * HPC + h) * VW:(kti * HPC + h + 1) * VW],
                            rhs=ex[:, hl * 512:(hl + 1) * 512],
                            start=(kti == 0),
                            stop=(kti == KT - 1),
                        )
                    if kti == KT - 1:
                        epi_stages.extend(make_epilogue(ot, m, qb))
                av_queue.append(av_stage)
                emit_slot()

    # drain pipeline
    while av_queue:
        av_queue.popleft()()
    while epi_stages:
        epi_stages.popleft()()


_NC_CACHE = {}


def _get_program(repeat=1, ablate=()):
    key = (repeat, tuple(ablate))
    if key not in _NC_CACHE:
        _NC_CACHE[key] = _build_program(repeat, ablate=tuple(ablate))
    return _NC_CACHE[key]


def _make_in_maps(x, mask, Wq, Wk, Wv):
    in_maps = []
    for core in range(NCORES):
        b, hg = core // 2, core % 2
        hsl = slice(hg * DQ, (hg + 1) * DQ)
        in_maps.append({
            "xT": np.ascontiguousarray(x[b].T).astype(NPBF16),
            "wqT": np.ascontiguousarray(Wq[hsl, :].T).astype(NPBF16),
            "wkT": np.ascontiguousarray(Wk[hsl, :].T).astype(NPBF16),
            "wvT": np.ascontiguousarray(Wv[hsl, :].T).astype(NPBF16),
            "nmT": np.ascontiguousarray((~mask[0, b]).T).astype(NPBF16),
        })
    return in_maps


def _assemble(results):
    out = np.empty((B, N, C), dtype=np.float32)
    for core in range(NCORES):
        b, hg = core // 2, core % 2
        out[b, :, hg * DQ:(hg + 1) * DQ] = results[core]["o"]
    return out


def run(x, mask, Wq, Wk, Wv, repeat=1, ablate=(), **spmd_kwargs):
    nc = _get_program(repeat, ablate=ablate)
    in_maps = _make_in_maps(
        np.asarray(x), np.asarray(mask), np.asarray(Wq), np.asarray(Wk), np.asarray(Wv)
    )
    res = run_bass_kernel_spmd(nc, in_maps, list(range(NCORES)), **spmd_kwargs)
    return _assemble(res.results), res


def kernel(x, mask, Wq, Wk, Wv):
    out, _ = run(x, mask, Wq, Wk, Wv)
    return out

